# revision 26
# baseline (speedup 1.0000x reference)
"""Trainium2 SPMD kernel for nn_CombinedGeneModel.

Math (per batch b, tech t, gene g; R = T*G independent tiny MLPs):
    h   = relu(x * w1[r,e] + b1[r,e])          e = 0..3
    s   = relu(sum_e h*w2[r,e] + b2[r])
    out = relu(sum_t s[b,t,g]*wg[g,t] + bg[g])

With b1 == 0 and b2 == 0 (guaranteed by setup_inputs) the whole network
folds into ONE per-gene 4-term MAC:
    sum_e w2_e*relu(w1_e*x) = a*relu(x) + e*relu(-x)
        a = sum_e w2_e*max(w1_e,0),  e = sum_e w2_e*max(-w1_e,0)
    s = relu(a*p+ + e*p-) = relu(a)*p+ + relu(e)*p-   (disjoint supports)
    wg*s = A*p+ + E*p-   with A = wg*relu(a), E = wg*relu(e)
         = F*|x| + G*x   with F = (A+E)/2,   G = (A-E)/2
so  out[g,b] = relu(F0|x0| + G0 x0 + F1|x1| + G1 x1 + bg).

Layout: genes on SBUF partitions, batch on the free axis; genes sharded
across the 8 NeuronCores; host pre-transposes x to [G, T, B] fp16 so all
DMA is contiguous.  Per 128-gene tile (free dim = 1024 batch):
  DVE    : diag stationaries diag(F0,G0,F1,G1) built on-chip from the
           [P,1] coefficient columns via identity*scalar; |x0| abs
  GpSimd : |x1| abs
  TensorE: 8 diag matmuls accumulate F0|x0|+G0x0+G1x1+F1|x1| in PSUM
  ScalarE: relu(PSUM + bg) -> output staging fp16; chunked HWDGE stores
Loads ride qSyncDynamicHW, stores qScalarDynamicHW, so the store stream
overlaps the load stream instead of serializing after it.

The walrus build here accepts at most ONE sync wait per instruction, so
buffers for DMA targets are no-reuse (or touch-scribbled on recycle),
dummy ldweights absorb cross-engine waits ahead of the matmul group, and
a post-pass splits any remaining multi-wait instruction (the epilogue
Drain)."""

import os
import numpy as np

N_GENES = 20000
N_TECH = 2
BATCH = 1024
N_CORES = 8
P = 128
G_PAD = 20480            # next multiple of 8*128 above 20000
GS = G_PAD // N_CORES    # 2560 genes per core
NTILES = GS // P         # 20 tiles of 128 genes
FD = BATCH               # free dim per (tile, tech)
HF = FD // 2             # matmul moving-free-dim limit is 512
NCOL = 5                 # per-gene scalars: A0, S0, A1, S1, bg
X_BUFS = NTILES          # x staging: no reuse, keeps the load DMAs waitless
ABS_BUFS = 16            # relu(-x) staging rings; recycle distance 16 is
                         # beyond any physically reachable engine skew
STORE_EVERY = 2          # output store granularity in tiles

LAST_EXEC_NS = None
LAST_RESULTS = None

_nc_cache = {}


def _build_nc():
    import concourse.bass as bass
    import concourse.mybir as mybir
    from concourse.tile import TileContext

    Op = mybir.AluOpType
    Act = mybir.ActivationFunctionType
    f16 = mybir.dt.float16
    f32 = mybir.dt.float32

    nc = bass.Bass()
    x_d = nc.declare_dram_parameter("x", [NTILES, P, 2 * FD], f16, isOutput=False)
    w_d = nc.declare_dram_parameter("w", [P, NTILES * NCOL], f32, isOutput=False)
    e_d = nc.declare_dram_parameter("eye", [P, P], f16, isOutput=False)
    # paired-tile layout: each store chunk is one contiguous 4KB line per
    # partition (2KB lines ran the store queue at only ~134 GB/s)
    o_d = nc.declare_dram_parameter("out", [NTILES // STORE_EVERY, P,
                                            STORE_EVERY * FD], f16,
                                    isOutput=True)

    with TileContext(nc) as tc:
        with (
            tc.tile_pool(name="wp", bufs=1) as wpool,
            tc.tile_pool(name="xp", bufs=X_BUFS) as xpool,
            tc.tile_pool(name="dp", bufs=NTILES) as dpool,
            tc.tile_pool(name="p0m", bufs=ABS_BUFS) as p0mpool,
            tc.tile_pool(name="p1m", bufs=ABS_BUFS) as p1mpool,
            tc.tile_pool(name="op", bufs=1) as opool,
            tc.tile_pool(name="ps", bufs=4, space="PSUM") as pspool,
        ):
            w = wpool.tile([P, NTILES * NCOL], f32)
            nc.sync.dma_start(w[:], w_d[:])
            eye = wpool.tile([P, P], f16)
            nc.sync.dma_start(eye[:], e_d[:])

            obuf = opool.tile([P, NTILES * FD], f16)

            # per-engine tiny tiles that absorb the w-DMA wait once and
            # serve as touch-scribble sources for buffer recycling
            wt_v = wpool.tile([P, 1], f16)
            nc.vector.tensor_copy(wt_v[:], w[:, 0:1])
            wt_a = wpool.tile([P, 1], f32)
            nc.scalar.copy(wt_a[:], w[:, 0:1])

            def touch(engine_copy, ring, src):
                # scribble: a tiny same-engine write to the recycled ring
                # slot's LAST element carries the WAR wait on the slot's
                # highest PE reader (the second-half matmul)
                engine_copy(ring[:, FD - 1 : FD], src[:])

            p0ms, p1ms = [], []
            for j in range(NTILES):
                xt = xpool.tile([P, 2 * FD], f16, tag="x")
                nc.sync.dma_start(xt[:], x_d[j])

                col = j * NCOL
                bg = w[:, col + 4 : col + 5]

                # DVE: diag stationaries (need only w, run during DMA),
                # then p0- = relu(-x0), p1- = relu(-x1).  Using
                #   A p+ + E p-  ==  A x + (A+E) p-
                # the raw x feeds the PE directly (no-reuse buffer), so
                # only the two p- streams need staging rings.
                dg = dpool.tile([P, 4 * P], f16, tag="dg")
                for k in range(4):
                    nc.vector.tensor_scalar(
                        dg[:, k * P : (k + 1) * P], eye[:],
                        w[:, col + k : col + k + 1], None, Op.mult,
                    )
                if j >= ABS_BUFS:
                    touch(nc.vector.tensor_copy, p0ms[j - ABS_BUFS], wt_v)
                a0m = p0mpool.tile([P, FD], f16, tag="a0m")
                p0ms.append(a0m)
                nc.vector.tensor_scalar(a0m[:], xt[:, 0:FD], -1.0, 0.0,
                                        Op.mult, Op.max)
                if j >= ABS_BUFS:
                    touch(nc.vector.tensor_copy, p1ms[j - ABS_BUFS], wt_v)
                a1m = p1mpool.tile([P, FD], f16, tag="a1m")
                p1ms.append(a1m)
                nc.vector.tensor_scalar(a1m[:], xt[:, FD : 2 * FD], -1.0, 0.0,
                                        Op.mult, Op.max)

                # TensorE: ps = A0 x0 + S0 p0- + A1 x1 + S1 p1- (PSUM f32).
                # two dummy ldweights absorb the DVE and DMA-lane waits so
                # each matmul carries at most one (mm1: PSUM-WAR)
                nc.tensor.ldweights(a1m[:, 0:P])
                nc.tensor.ldweights(xt[:, 0:P])
                dgA0 = dg[:, 0 * P : 1 * P]
                dgS0 = dg[:, 1 * P : 2 * P]
                dgA1 = dg[:, 2 * P : 3 * P]
                dgS1 = dg[:, 3 * P : 4 * P]
                ps = pspool.tile([P, FD], f32, tag="ps")
                for h in range(2):
                    sl = slice(h * HF, (h + 1) * HF)
                    nc.tensor.matmul(ps[:, sl], dgA0, xt[:, sl],
                                     start=True, stop=False)
                for h in range(2):
                    sl = slice(h * HF, (h + 1) * HF)
                    nc.tensor.matmul(ps[:, sl], dgS0, a0m[:, sl],
                                     start=False, stop=False)
                for h in range(2):
                    sl = slice(h * HF, (h + 1) * HF)
                    nc.tensor.matmul(ps[:, sl], dgA1,
                                     xt[:, FD + h * HF : FD + (h + 1) * HF],
                                     start=False, stop=False)
                for h in range(2):
                    sl = slice(h * HF, (h + 1) * HF)
                    nc.tensor.matmul(ps[:, sl], dgS1, a1m[:, sl],
                                     start=False, stop=True)

                # ScalarE: relu(ps + bg) -> staging fp16 (one activation
                # per PSUM bank: an op must not cross the 2KB boundary),
                # then the chunked HWDGE store (self-ordered after the
                # activations; overlaps the load stream)
                for h in range(2):
                    nc.scalar.activation(
                        obuf[:, j * FD + h * HF : j * FD + (h + 1) * HF],
                        ps[:, h * HF : (h + 1) * HF], Act.Relu, bias=bg)
                # Store routing: a single store queue self-caps at ~160
                # GB/s, which paces the whole kernel.  Alternate chunks
                # between the Scalar HWDGE queue and the (otherwise idle)
                # GpSimd SWDGE queue; the last two tiles go out singly,
                # one on qSync (its load descriptors have drained by
                # then), so the post-drain backlog rides three queues.
                if j >= 18:
                    eng = nc.sync if j % 2 == 0 else nc.scalar
                    c, half = j // STORE_EVERY, j % STORE_EVERY
                    eng.dma_start(o_d[c][:, half * FD : (half + 1) * FD],
                                  obuf[:, j * FD : (j + 1) * FD])
                elif (j + 1) % STORE_EVERY == 0:
                    c = j // STORE_EVERY
                    eng = nc.scalar if c % 2 == 0 else nc.gpsimd
                    eng.dma_start(o_d[c],
                                  obuf[:, (j - 1) * FD : (j + 1) * FD])

    _split_multi_waits(nc, mybir)
    return nc


def _split_multi_waits(nc, mybir):
    """walrus (gen3 codegen here) accepts at most one sync wait per
    instruction.  Two rewrites keep every instruction at <=1 wait:

    1. Drop self-engine waits that are provably satisfied: engines run
       their stream in order and bump their own semaphore once per
       retired instruction, so a wait on the engine's own semaphore for
       a value already reached earlier in its own stream is a no-op
       (Tile emits these because its clock tracking is not transitive).
    2. For the remaining multi-wait instructions (the epilogue Drain,
       which is block-initial), hoist all but one wait onto same-engine
       NoOps appended to the preceding basic block."""
    blocks = list(nc.main_func.blocks)

    # sem id -> set of engines that increment it
    updaters = {}
    for bb in blocks:
        for ins in bb.instructions:
            si = getattr(ins, "sync_info", None)
            if si is None:
                continue
            for up in si.on_update or []:
                updaters.setdefault(up.id, set()).add(ins.engine)

    # pass 1: strip satisfied self-waits, walking in block order while
    # accumulating each semaphore's increments
    cum = {}
    for bb in blocks:
        for ins in bb.instructions:
            si = getattr(ins, "sync_info", None)
            if si is None:
                continue
            waits = list(si.on_wait or [])
            if len(waits) > 1:
                kept = []
                for wv in waits:
                    if (
                        wv.sync_type == "semaphore"
                        and wv.wait_mode == "sem-ge-imm"
                        and updaters.get(wv.id) == {ins.engine}
                        # engine sems increment at in-order instruction
                        # retirement, so earlier-stream increments prove the
                        # wait satisfied; DMA lane sems (DMAHW*/DMASW*)
                        # increment at async DMA *completion* — never strip
                        and "DMA" not in (wv.ant_name or "")
                        and cum.get(wv.id, 0) >= wv.wait_value
                    ):
                        continue  # provably satisfied self-wait
                    kept.append(wv)
                if len(kept) != len(waits):
                    ins.sync_info = mybir.SyncInfo(
                        on_wait=kept, on_update=list(si.on_update or [])
                    )
            si = ins.sync_info
            for up in si.on_update or []:
                if up.update_mode == "sem-inc":
                    cum[up.id] = cum.get(up.id, 0) + up.update_value

    # pass 2: NoOp-split anything still multi-wait (the epilogue Drain,
    # plus the occasional scheduler-reordered data op).  Same-engine
    # NoOps inserted immediately BEFORE the instruction carry all but
    # one wait; walrus lowers each engine's program in block-list order,
    # so the engine stream ordering is preserved.
    nop_idx = 0
    for bb in blocks:
        insns = bb.instructions
        pending = []
        for idx, ins in enumerate(insns):
            si = getattr(ins, "sync_info", None)
            if si is None:
                continue
            waits = list(si.on_wait or [])
            if len(waits) <= 1:
                continue
            pending.append((idx, ins, waits))
        for idx, ins, waits in reversed(pending):
            nops = []
            for wv in waits[:-1]:
                nop = mybir.InstNoOp(name=f"ant-waitsplit-{nop_idx}")
                nop_idx += 1
                nop.engine = ins.engine
                nop.sync_info = mybir.SyncInfo(on_wait=[wv], on_update=[])
                nops.append(nop)
            ins.sync_info = mybir.SyncInfo(
                on_wait=[waits[-1]],
                on_update=list(ins.sync_info.on_update or []),
            )
            bb.instructions = insns[:idx] + nops + insns[idx:]
            insns = bb.instructions


def _numpy_fallback(x, w1, b1, w2, b2, wg, bgv):
    B = x.shape[0]
    R = N_GENES * N_TECH
    xr = x.reshape(B, R).T.astype(np.float32)
    h = np.maximum(xr[:, :, None] * w1[:, None, :] + b1[:, None, :], 0.0)
    s = np.maximum(np.einsum("rbe,re->rb", h, w2) + b2[:, None], 0.0)
    s = s.T.reshape(B, N_TECH, N_GENES)
    out = np.maximum(np.einsum("btg,gt->bg", s, wg) + bgv, 0.0)
    return out.astype(np.float32)


def kernel(x, weights1, bias1, weights2, bias2, weights_g, bias_g):
    global LAST_EXEC_NS, LAST_RESULTS
    x = np.asarray(x, dtype=np.float32)
    w1 = np.asarray(weights1, dtype=np.float32)
    b1 = np.asarray(bias1, dtype=np.float32)
    w2 = np.asarray(weights2, dtype=np.float32)
    b2 = np.asarray(bias2, dtype=np.float32)
    wg = np.asarray(weights_g, dtype=np.float32)
    bgv = np.asarray(bias_g, dtype=np.float32)

    if np.any(b1 != 0.0) or np.any(b2 != 0.0):
        # the relu folding below needs b1 == b2 == 0; exact general fallback
        return _numpy_fallback(x, w1, b1, w2, b2, wg, bgv)

    G = N_GENES
    # fold both relu stages into per-gene relu(x)/relu(-x) coefficients
    a = (w2 * np.maximum(w1, 0.0)).sum(axis=1)   # [R]
    e = (w2 * np.maximum(-w1, 0.0)).sum(axis=1)  # [R]
    wgr = wg.T.reshape(-1)                       # row r: tech r//G, gene r%G
    A = wgr * np.maximum(a, 0.0)
    E = wgr * np.maximum(e, 0.0)
    S = A + E          # A p+ + E p-  ==  A x + S relu(-x)

    # per-gene scalar table [G_PAD, NCOL]: A0, S0, A1, S1, bg
    wtab = np.zeros((G_PAD, NCOL), dtype=np.float32)
    wtab[:G, 0] = A[:G]
    wtab[:G, 1] = S[:G]
    wtab[:G, 2] = A[G:]
    wtab[:G, 3] = S[G:]
    wtab[:G, 4] = bgv

    # x -> [G_PAD, T, B] fp16, contiguous per gene row
    xt = np.zeros((G_PAD, N_TECH, BATCH), dtype=np.float16)
    xt[:G] = x.transpose(2, 1, 0)

    eye = np.eye(P, dtype=np.float16)
    in_maps = []
    for i in range(N_CORES):
        g0 = i * GS
        xi = np.ascontiguousarray(xt[g0 : g0 + GS].reshape(NTILES, P, 2 * FD))
        wi = np.ascontiguousarray(
            wtab[g0 : g0 + GS].reshape(NTILES, P, NCOL).transpose(1, 0, 2)
            .reshape(P, NTILES * NCOL)
        )
        in_maps.append({"x": xi, "w": wi, "eye": eye})

    if "nc" not in _nc_cache:
        _nc_cache["nc"] = _build_nc()
    nc = _nc_cache["nc"]

    from concourse.bass_utils import run_bass_kernel_spmd

    trace = bool(int(os.environ.get("KERNEL_TRACE", "0")))
    res = run_bass_kernel_spmd(nc, in_maps, core_ids=list(range(N_CORES)),
                               trace=trace)
    LAST_EXEC_NS = res.exec_time_ns
    LAST_RESULTS = res

    # out is [NTILES//2, P, 2, FD]: chunk c, partition p, tile 2c+t
    parts = [
        res.results[i]["out"]
        .reshape(NTILES // STORE_EVERY, P, STORE_EVERY, BATCH)
        .transpose(0, 2, 1, 3)
        .reshape(GS, BATCH)
        for i in range(N_CORES)
    ]
    full = np.concatenate(parts, axis=0)[:G]          # [G, B] fp16
    return np.ascontiguousarray(full.T).astype(np.float32)


# revision 27
# speedup vs baseline: 1.0946x; 1.0946x over previous
"""Trainium2 SPMD kernel for nn_CombinedGeneModel.

Math (per batch b, tech t, gene g; R = T*G independent tiny MLPs):
    h   = relu(x * w1[r,e] + b1[r,e])          e = 0..3
    s   = relu(sum_e h*w2[r,e] + b2[r])
    out = relu(sum_t s[b,t,g]*wg[g,t] + bg[g])

With b1 == 0 and b2 == 0 (guaranteed by setup_inputs) the whole network
folds into ONE per-gene 4-term MAC:
    sum_e w2_e*relu(w1_e*x) = a*relu(x) + e*relu(-x)
        a = sum_e w2_e*max(w1_e,0),  e = sum_e w2_e*max(-w1_e,0)
    s = relu(a*p+ + e*p-) = relu(a)*p+ + relu(e)*p-   (disjoint supports)
    wg*s = A*p+ + E*p-   with A = wg*relu(a), E = wg*relu(e)
         = F*|x| + G*x   with F = (A+E)/2,   G = (A-E)/2
so  out[g,b] = relu(F0|x0| + G0 x0 + F1|x1| + G1 x1 + bg).

Layout: genes on SBUF partitions, batch on the free axis; genes sharded
across the 8 NeuronCores; host pre-transposes x to [G, T, B] fp16 so all
DMA is contiguous.  Per 128-gene tile (free dim = 1024 batch):
  DVE    : diag stationaries diag(F0,G0,F1,G1) built on-chip from the
           [P,1] coefficient columns via identity*scalar; |x0| abs
  GpSimd : |x1| abs
  TensorE: 8 diag matmuls accumulate F0|x0|+G0x0+G1x1+F1|x1| in PSUM
  ScalarE: relu(PSUM + bg) -> output staging fp16; chunked HWDGE stores
Loads ride qSyncDynamicHW, stores qScalarDynamicHW, so the store stream
overlaps the load stream instead of serializing after it.

The walrus build here accepts at most ONE sync wait per instruction, so
buffers for DMA targets are no-reuse (or touch-scribbled on recycle),
dummy ldweights absorb cross-engine waits ahead of the matmul group, and
a post-pass splits any remaining multi-wait instruction (the epilogue
Drain)."""

import os
import numpy as np

N_GENES = 20000
N_TECH = 2
BATCH = 1024
N_CORES = 8
P = 128
G_PAD = 20480            # next multiple of 8*128 above 20000
GS = G_PAD // N_CORES    # 2560 genes per core
NTILES = GS // P         # 20 tiles of 128 genes
FD = BATCH               # free dim per (tile, tech)
HF = FD // 2             # matmul moving-free-dim limit is 512
NCOL = 5                 # per-gene scalars: A0, S0, A1, S1, bg
X_BUFS = NTILES          # x staging: no reuse, keeps the load DMAs waitless
ABS_BUFS = 16            # relu(-x) staging rings; recycle distance 16 is
                         # beyond any physically reachable engine skew
STORE_EVERY = 2          # output store granularity in tiles

LAST_EXEC_NS = None
LAST_RESULTS = None

_nc_cache = {}


def _build_nc():
    import concourse.bass as bass
    import concourse.mybir as mybir
    from concourse.tile import TileContext

    Op = mybir.AluOpType
    Act = mybir.ActivationFunctionType
    f16 = mybir.dt.float16
    f32 = mybir.dt.float32

    nc = bass.Bass()
    x_d = nc.declare_dram_parameter("x", [NTILES, P, 2 * FD], f16, isOutput=False)
    w_d = nc.declare_dram_parameter("w", [P, NTILES * NCOL], f32, isOutput=False)
    e_d = nc.declare_dram_parameter("eye", [P, P], f16, isOutput=False)
    # paired-tile layout: each store chunk is one contiguous 4KB line per
    # partition (2KB lines ran the store queue at only ~134 GB/s)
    o_d = nc.declare_dram_parameter("out", [NTILES // STORE_EVERY, P,
                                            STORE_EVERY * FD], f16,
                                    isOutput=True)

    with TileContext(nc) as tc:
        with (
            tc.tile_pool(name="wp", bufs=1) as wpool,
            tc.tile_pool(name="xp", bufs=X_BUFS) as xpool,
            tc.tile_pool(name="dp", bufs=NTILES) as dpool,
            tc.tile_pool(name="p0m", bufs=ABS_BUFS) as p0mpool,
            tc.tile_pool(name="p1m", bufs=ABS_BUFS) as p1mpool,
            tc.tile_pool(name="op", bufs=1) as opool,
            tc.tile_pool(name="ps", bufs=4, space="PSUM") as pspool,
        ):
            w = wpool.tile([P, NTILES * NCOL], f32)
            nc.sync.dma_start(w[:], w_d[:])
            eye = wpool.tile([P, P], f16)
            nc.sync.dma_start(eye[:], e_d[:])

            obuf = opool.tile([P, NTILES * FD], f16)

            # per-engine tiny tiles that absorb the w-DMA wait once and
            # serve as touch-scribble sources for buffer recycling
            wt_v = wpool.tile([P, 1], f16)
            nc.vector.tensor_copy(wt_v[:], w[:, 0:1])
            wt_a = wpool.tile([P, 1], f32)
            nc.scalar.copy(wt_a[:], w[:, 0:1])

            def touch(engine_copy, ring, src):
                # scribble: a tiny same-engine write to the recycled ring
                # slot's LAST element carries the WAR wait on the slot's
                # highest PE reader (the second-half matmul)
                engine_copy(ring[:, FD - 1 : FD], src[:])

            p0ms, p1ms = [], []
            for j in range(NTILES):
                xt = xpool.tile([P, 2 * FD], f16, tag="x")
                nc.sync.dma_start(xt[:], x_d[j])

                col = j * NCOL
                bg = w[:, col + 4 : col + 5]

                # DVE: diag stationaries (need only w, run during DMA),
                # then p0- = relu(-x0), p1- = relu(-x1).  Using
                #   A p+ + E p-  ==  A x + (A+E) p-
                # the raw x feeds the PE directly (no-reuse buffer), so
                # only the two p- streams need staging rings.
                dg = dpool.tile([P, 4 * P], f16, tag="dg")
                for k in range(4):
                    nc.vector.tensor_scalar(
                        dg[:, k * P : (k + 1) * P], eye[:],
                        w[:, col + k : col + k + 1], None, Op.mult,
                    )
                if j >= ABS_BUFS:
                    touch(nc.vector.tensor_copy, p0ms[j - ABS_BUFS], wt_v)
                a0m = p0mpool.tile([P, FD], f16, tag="a0m")
                p0ms.append(a0m)
                nc.vector.tensor_scalar(a0m[:], xt[:, 0:FD], -1.0, 0.0,
                                        Op.mult, Op.max)
                if j >= ABS_BUFS:
                    touch(nc.vector.tensor_copy, p1ms[j - ABS_BUFS], wt_v)
                a1m = p1mpool.tile([P, FD], f16, tag="a1m")
                p1ms.append(a1m)
                nc.vector.tensor_scalar(a1m[:], xt[:, FD : 2 * FD], -1.0, 0.0,
                                        Op.mult, Op.max)

                # TensorE: ps = A0 x0 + S0 p0- + A1 x1 + S1 p1- (PSUM f32).
                # two dummy ldweights absorb the DVE and DMA-lane waits so
                # each matmul carries at most one (mm1: PSUM-WAR)
                nc.tensor.ldweights(a1m[:, 0:P])
                nc.tensor.ldweights(xt[:, 0:P])
                dgA0 = dg[:, 0 * P : 1 * P]
                dgS0 = dg[:, 1 * P : 2 * P]
                dgA1 = dg[:, 2 * P : 3 * P]
                dgS1 = dg[:, 3 * P : 4 * P]
                ps = pspool.tile([P, FD], f32, tag="ps")
                for h in range(2):
                    sl = slice(h * HF, (h + 1) * HF)
                    nc.tensor.matmul(ps[:, sl], dgA0, xt[:, sl],
                                     start=True, stop=False)
                for h in range(2):
                    sl = slice(h * HF, (h + 1) * HF)
                    nc.tensor.matmul(ps[:, sl], dgS0, a0m[:, sl],
                                     start=False, stop=False)
                for h in range(2):
                    sl = slice(h * HF, (h + 1) * HF)
                    nc.tensor.matmul(ps[:, sl], dgA1,
                                     xt[:, FD + h * HF : FD + (h + 1) * HF],
                                     start=False, stop=False)
                for h in range(2):
                    sl = slice(h * HF, (h + 1) * HF)
                    nc.tensor.matmul(ps[:, sl], dgS1, a1m[:, sl],
                                     start=False, stop=True)

                # ScalarE: relu(ps + bg) -> staging fp16 (one activation
                # per PSUM bank: an op must not cross the 2KB boundary),
                # then the chunked HWDGE store (self-ordered after the
                # activations; overlaps the load stream)
                for h in range(2):
                    nc.scalar.activation(
                        obuf[:, j * FD + h * HF : j * FD + (h + 1) * HF],
                        ps[:, h * HF : (h + 1) * HF], Act.Relu, bias=bg)
                # Store routing: paired chunks ride the Scalar HWDGE
                # queue; the last four tiles go out singly, alternating
                # onto qSync (its load descriptors have drained by then)
                # so the post-drain backlog is split across two queues.
                # (Routing stores via the GpSimd SWDGE queue concurrently
                # with the loads throttles the whole DMA subsystem.)
                if j >= 16:
                    eng = nc.sync if j % 2 == 0 else nc.scalar
                    c, half = j // STORE_EVERY, j % STORE_EVERY
                    eng.dma_start(o_d[c][:, half * FD : (half + 1) * FD],
                                  obuf[:, j * FD : (j + 1) * FD])
                elif (j + 1) % STORE_EVERY == 0:
                    nc.scalar.dma_start(o_d[j // STORE_EVERY],
                                        obuf[:, (j - 1) * FD : (j + 1) * FD])

    _split_multi_waits(nc, mybir)
    return nc


def _split_multi_waits(nc, mybir):
    """walrus (gen3 codegen here) accepts at most one sync wait per
    instruction.  Two rewrites keep every instruction at <=1 wait:

    1. Drop self-engine waits that are provably satisfied: engines run
       their stream in order and bump their own semaphore once per
       retired instruction, so a wait on the engine's own semaphore for
       a value already reached earlier in its own stream is a no-op
       (Tile emits these because its clock tracking is not transitive).
    2. For the remaining multi-wait instructions (the epilogue Drain,
       which is block-initial), hoist all but one wait onto same-engine
       NoOps appended to the preceding basic block."""
    blocks = list(nc.main_func.blocks)

    # sem id -> set of engines that increment it
    updaters = {}
    for bb in blocks:
        for ins in bb.instructions:
            si = getattr(ins, "sync_info", None)
            if si is None:
                continue
            for up in si.on_update or []:
                updaters.setdefault(up.id, set()).add(ins.engine)

    # pass 1: strip satisfied self-waits, walking in block order while
    # accumulating each semaphore's increments
    cum = {}
    for bb in blocks:
        for ins in bb.instructions:
            si = getattr(ins, "sync_info", None)
            if si is None:
                continue
            waits = list(si.on_wait or [])
            if len(waits) > 1:
                kept = []
                for wv in waits:
                    if (
                        wv.sync_type == "semaphore"
                        and wv.wait_mode == "sem-ge-imm"
                        and updaters.get(wv.id) == {ins.engine}
                        # engine sems increment at in-order instruction
                        # retirement, so earlier-stream increments prove the
                        # wait satisfied; DMA lane sems (DMAHW*/DMASW*)
                        # increment at async DMA *completion* — never strip
                        and "DMA" not in (wv.ant_name or "")
                        and cum.get(wv.id, 0) >= wv.wait_value
                    ):
                        continue  # provably satisfied self-wait
                    kept.append(wv)
                if len(kept) != len(waits):
                    ins.sync_info = mybir.SyncInfo(
                        on_wait=kept, on_update=list(si.on_update or [])
                    )
            si = ins.sync_info
            for up in si.on_update or []:
                if up.update_mode == "sem-inc":
                    cum[up.id] = cum.get(up.id, 0) + up.update_value

    # pass 2: NoOp-split anything still multi-wait (the epilogue Drain,
    # plus the occasional scheduler-reordered data op).  Same-engine
    # NoOps inserted immediately BEFORE the instruction carry all but
    # one wait; walrus lowers each engine's program in block-list order,
    # so the engine stream ordering is preserved.
    nop_idx = 0
    for bb in blocks:
        insns = bb.instructions
        pending = []
        for idx, ins in enumerate(insns):
            si = getattr(ins, "sync_info", None)
            if si is None:
                continue
            waits = list(si.on_wait or [])
            if len(waits) <= 1:
                continue
            pending.append((idx, ins, waits))
        for idx, ins, waits in reversed(pending):
            nops = []
            for wv in waits[:-1]:
                nop = mybir.InstNoOp(name=f"ant-waitsplit-{nop_idx}")
                nop_idx += 1
                nop.engine = ins.engine
                nop.sync_info = mybir.SyncInfo(on_wait=[wv], on_update=[])
                nops.append(nop)
            ins.sync_info = mybir.SyncInfo(
                on_wait=[waits[-1]],
                on_update=list(ins.sync_info.on_update or []),
            )
            bb.instructions = insns[:idx] + nops + insns[idx:]
            insns = bb.instructions


def _numpy_fallback(x, w1, b1, w2, b2, wg, bgv):
    B = x.shape[0]
    R = N_GENES * N_TECH
    xr = x.reshape(B, R).T.astype(np.float32)
    h = np.maximum(xr[:, :, None] * w1[:, None, :] + b1[:, None, :], 0.0)
    s = np.maximum(np.einsum("rbe,re->rb", h, w2) + b2[:, None], 0.0)
    s = s.T.reshape(B, N_TECH, N_GENES)
    out = np.maximum(np.einsum("btg,gt->bg", s, wg) + bgv, 0.0)
    return out.astype(np.float32)


def kernel(x, weights1, bias1, weights2, bias2, weights_g, bias_g):
    global LAST_EXEC_NS, LAST_RESULTS
    x = np.asarray(x, dtype=np.float32)
    w1 = np.asarray(weights1, dtype=np.float32)
    b1 = np.asarray(bias1, dtype=np.float32)
    w2 = np.asarray(weights2, dtype=np.float32)
    b2 = np.asarray(bias2, dtype=np.float32)
    wg = np.asarray(weights_g, dtype=np.float32)
    bgv = np.asarray(bias_g, dtype=np.float32)

    if np.any(b1 != 0.0) or np.any(b2 != 0.0):
        # the relu folding below needs b1 == b2 == 0; exact general fallback
        return _numpy_fallback(x, w1, b1, w2, b2, wg, bgv)

    G = N_GENES
    # fold both relu stages into per-gene relu(x)/relu(-x) coefficients
    a = (w2 * np.maximum(w1, 0.0)).sum(axis=1)   # [R]
    e = (w2 * np.maximum(-w1, 0.0)).sum(axis=1)  # [R]
    wgr = wg.T.reshape(-1)                       # row r: tech r//G, gene r%G
    A = wgr * np.maximum(a, 0.0)
    E = wgr * np.maximum(e, 0.0)
    S = A + E          # A p+ + E p-  ==  A x + S relu(-x)

    # per-gene scalar table [G_PAD, NCOL]: A0, S0, A1, S1, bg
    wtab = np.zeros((G_PAD, NCOL), dtype=np.float32)
    wtab[:G, 0] = A[:G]
    wtab[:G, 1] = S[:G]
    wtab[:G, 2] = A[G:]
    wtab[:G, 3] = S[G:]
    wtab[:G, 4] = bgv

    # x -> [G_PAD, T, B] fp16, contiguous per gene row
    xt = np.zeros((G_PAD, N_TECH, BATCH), dtype=np.float16)
    xt[:G] = x.transpose(2, 1, 0)

    eye = np.eye(P, dtype=np.float16)
    in_maps = []
    for i in range(N_CORES):
        g0 = i * GS
        xi = np.ascontiguousarray(xt[g0 : g0 + GS].reshape(NTILES, P, 2 * FD))
        wi = np.ascontiguousarray(
            wtab[g0 : g0 + GS].reshape(NTILES, P, NCOL).transpose(1, 0, 2)
            .reshape(P, NTILES * NCOL)
        )
        in_maps.append({"x": xi, "w": wi, "eye": eye})

    if "nc" not in _nc_cache:
        _nc_cache["nc"] = _build_nc()
    nc = _nc_cache["nc"]

    from concourse.bass_utils import run_bass_kernel_spmd

    trace = bool(int(os.environ.get("KERNEL_TRACE", "0")))
    res = run_bass_kernel_spmd(nc, in_maps, core_ids=list(range(N_CORES)),
                               trace=trace)
    LAST_EXEC_NS = res.exec_time_ns
    LAST_RESULTS = res

    # out is [NTILES//2, P, 2, FD]: chunk c, partition p, tile 2c+t
    parts = [
        res.results[i]["out"]
        .reshape(NTILES // STORE_EVERY, P, STORE_EVERY, BATCH)
        .transpose(0, 2, 1, 3)
        .reshape(GS, BATCH)
        for i in range(N_CORES)
    ]
    full = np.concatenate(parts, axis=0)[:G]          # [G, B] fp16
    return np.ascontiguousarray(full.T).astype(np.float32)
